# revision 9
# baseline (speedup 1.0000x reference)
"""Trainium2 Bass kernel for nn_CrossAttnMLP (cross-attn + dual FFN + BN MLP head).

Sharding: pure data-parallel over 8 NeuronCores (batch 65536 -> 8 x 8192).

On-chip layout keeps features on the SBUF partition dim and batch on the free
dim, so every layer is matmul(lhsT=W^T, rhs=act) and layers chain with no
transposes; x is pre-transposed (and feature-padded to 896) on the host.
LayerNorm runs via PE projector matmuls: diff = (I - 11^T/128) @ z and
var = (11^T/128) @ diff^2, then r = rsqrt(var+eps) on ScalarE and a single
fused (diff*g)*r on VectorE. All affine biases are folded host-side into
per-partition bias vectors applied inside fused ACT/DVE ops.
BatchNorm uses exact full-batch stats: per-core sum/sumsq accumulate free via
activation accum_out, then one tiny AllReduce per BN layer (128x2 / 64x2).
Matmuls run in float32r (TF32, 1 cycle/row at N>=256) with fp32 PSUM.
"""
import sys, os
sys.path.insert(0, "/opt/trn_rl_repo")
import numpy as np
import concourse.bass as bass
import concourse.bacc as bacc
import concourse.tile as tile
from concourse import mybir
from concourse.bass_utils import run_bass_kernel_spmd

AF = mybir.ActivationFunctionType
ALU = mybir.AluOpType
F32 = mybir.dt.float32
F32R = mybir.dt.float32r

N_CORES = 8
B = 65536
PEP, TCR, D, FF = 384, 480, 128, 512
H1, H2 = 128, 64
EPS = 1e-5
XP = 896            # padded x feature dim (7 x 128)
NK = XP // 128      # 7 x-chunks
BC = B // N_CORES   # 8192 rows per core
N = 512             # batch columns per tile
NT = BC // N        # 16 tiles per core

# vecs ([128, 12] fp32) column indices (C_QC2*: Q @ (ffn_b2 + ln_b1) fold)
(C_BZ1P, C_BZ1T, C_G1P, C_G1T, C_QC2P, C_QC2T, C_G2P, C_G2T,
 C_BH1, C_BN1G, C_BN1B, C_PAD) = range(12)
# vech ([64, 4] fp32): 0=b_h2, 1=bn2_g, 2=bn2_b, 3=b_out(at row 0)

LAST_RESULT = None
_NC_CACHE = {}


def _build(single=False):
    nc = bacc.Bacc("TRN2", target_bir_lowering=False, debug=False,
                   enable_asserts=True, num_devices=(1 if single else N_CORES))

    def din(name, shape, dt=F32R):
        return nc.dram_tensor(name, shape, dt, kind="ExternalInput").ap()

    xt_d = din("xt", [XP, BC])
    wpep_d = din("wpepT", [PEP, D])
    wtcr_d = din("wtcrT", [512, D])            # padded 480 -> 512
    wap_d = din("wattnpT", [D, D])
    wat_d = din("wattntT", [D, D])
    q_d = din("qT", [D, D])
    pm_d = din("pT", [D, D])
    w1p_d = din("w1pT", [D, FF])
    w1t_d = din("w1tT", [D, FF])
    w2p_d = din("w2pT", [FF, D])
    w2t_d = din("w2tT", [FF, D])
    qg1p_d = din("qg1pT", [D, D])
    qg1t_d = din("qg1tT", [D, D])
    wh1p_d = din("wh1pT", [D, H1])
    wh1t_d = din("wh1tT", [D, H1])
    wh2_d = din("wh2T", [H1, H2])
    wout_d = din("woutT", [H2, 1])
    vecs_d = din("vecs", [D, 12], F32)
    vech_d = din("vech", [H2, 5], F32)
    bf1p_d = din("bf1p", [D, 4], F32)
    bf1t_d = din("bf1t", [D, 4], F32)
    y_d = nc.dram_tensor("y", [1, BC], F32, kind="ExternalOutput").ap()

    with tile.TileContext(nc) as tc:
        with tc.tile_pool(name="wpool", bufs=1) as wp, \
             tc.tile_pool(name="xpool", bufs=2) as xp, \
             tc.tile_pool(name="work", bufs=2) as wk, \
             tc.tile_pool(name="keep", bufs=1) as kp, \
             tc.tile_pool(name="ps1", bufs=1, space="PSUM") as ps1, \
             tc.tile_pool(name="ps2", bufs=2, space="PSUM") as ps2, \
             tc.tile_pool(name="dram", bufs=1, space="DRAM") as dr:

            # ---- load weights (once) ----
            def wtile(dram_ap, shape, tag, dt=F32R):
                t = wp.tile(shape, dt, tag=tag)
                nc.sync.dma_start(t[:], dram_ap)
                return t
            wpep = wtile(wpep_d.rearrange("(k p) m -> p k m", p=128), [128, 3, D], "wpep")
            wtcr = wtile(wtcr_d.rearrange("(k p) m -> p k m", p=128), [128, 4, D], "wtcr")
            wap = wtile(wap_d[:], [D, D], "wap")
            wat = wtile(wat_d[:], [D, D], "wat")
            qm = wtile(q_d[:], [D, D], "qm")
            pm = wtile(pm_d[:], [D, D], "pm")
            w1p = wtile(w1p_d[:], [D, FF], "w1p")
            w1t = wtile(w1t_d[:], [D, FF], "w1t")
            w2p = wtile(w2p_d.rearrange("(k p) m -> p k m", p=128), [128, 4, D], "w2p")
            w2t = wtile(w2t_d.rearrange("(k p) m -> p k m", p=128), [128, 4, D], "w2t")
            qg1p = wtile(qg1p_d[:], [D, D], "qg1p")
            qg1t = wtile(qg1t_d[:], [D, D], "qg1t")
            wh1p = wtile(wh1p_d[:], [D, H1], "wh1p")
            wh1t = wtile(wh1t_d[:], [D, H1], "wh1t")
            wh2 = wtile(wh2_d[:], [H1, H2], "wh2")
            wout = wtile(wout_d[:], [H2, 1], "wout")
            vecs = wtile(vecs_d[:], [D, 12], "vecs", F32)
            vech = wtile(vech_d[:], [H2, 5], "vech", F32)
            bf1p = wtile(bf1p_d[:], [D, 4], "bf1p", F32)
            bf1t = wtile(bf1t_d[:], [D, 4], "bf1t", F32)

            def vcol(c):
                return vecs[:, c:c + 1]

            # ---- retained activations + per-tile stats columns ----
            h1pre = kp.tile([D, NT, N], F32R, tag="h1pre")
            h2pre = kp.tile([H2, NT, N], F32R, tag="h2pre")
            s1c = kp.tile([D, NT], F32, tag="s1c")
            s2c = kp.tile([D, NT], F32, tag="s2c")
            u1c = kp.tile([H2, NT], F32, tag="u1c")
            u2c = kp.tile([H2, NT], F32, tag="u2c")

            xt_r = xt_d.rearrange("(k p) n -> p k n", p=128)

            # =================== phase A ===================
            for i in range(NT):
                xt = xp.tile([128, NK, N], F32R, tag="xt")
                nc.sync.dma_start(xt[:], xt_r[:, :, i * N:(i + 1) * N])

                # front accumulators share one 2-bank psum tile: [:,0,:]=t, [:,1,:]=p
                fr = ps2.tile([D, 2, N], F32, tag="scratchA")
                t_ps = fr[:, 0, :]
                p_ps = fr[:, 1, :]
                for k in range(4):
                    nc.tensor.matmul(t_ps, wtcr[:, k, :], xt[:, 3 + k, :],
                                     start=(k == 0), stop=False)
                tcr = wk.tile([D, N], F32R, tag="tcr")
                nc.vector.tensor_copy(tcr[:], t_ps)
                for k in range(3):
                    nc.tensor.matmul(p_ps, wpep[:, k, :], xt[:, k, :],
                                     start=(k == 0), stop=False)
                pep = wk.tile([D, N], F32R, tag="pep")
                nc.vector.tensor_copy(pep[:], p_ps)
                nc.tensor.matmul(p_ps, wap[:], tcr[:], start=False, stop=True)
                nc.tensor.matmul(t_ps, wat[:], pep[:], start=False, stop=True)

                # biased z1 pair in one SBUF tile
                z1 = wk.tile([D, 2, N], F32R, tag="z1")
                nc.vector.tensor_scalar_add(z1[:, 0, :], p_ps, vcol(C_BZ1P))
                nc.vector.tensor_scalar_add(z1[:, 1, :], t_ps, vcol(C_BZ1T))

                # LN1: diff pair, var pair, r pair
                diff1 = ps1.tile([D, 2, N], F32, tag="diff1")
                nc.tensor.matmul(diff1[:, 0, :], qm[:], z1[:, 0, :], start=True, stop=True)
                nc.tensor.matmul(diff1[:, 1, :], qm[:], z1[:, 1, :], start=True, stop=True)
                dsq1 = wk.tile([D, 2, N], F32R, tag="dsq1")
                nc.scalar.activation(dsq1[:], diff1[:], AF.Square)
                var1 = ps2.tile([D, 2, N], F32, tag="scratchA")
                nc.tensor.matmul(var1[:, 0, :], pm[:], dsq1[:, 0, :], start=True, stop=True)
                nc.tensor.matmul(var1[:, 1, :], pm[:], dsq1[:, 1, :], start=True, stop=True)
                r1 = wk.tile([D, 2, N], F32, tag="r1")
                nc.scalar.activation(r1[:], var1[:], AF.Abs_reciprocal_sqrt,
                                     bias=vcol(C_PAD))
                ln1p = wk.tile([D, N], F32R, tag="ln1p")
                nc.vector.scalar_tensor_tensor(
                    ln1p[:], diff1[:, 0, :], vcol(C_G1P), r1[:, 0, :],
                    op0=ALU.mult, op1=ALU.mult)
                ln1t = wk.tile([D, N], F32R, tag="ln1t")
                nc.vector.scalar_tensor_tensor(
                    ln1t[:], diff1[:, 1, :], vcol(C_G1T), r1[:, 1, :],
                    op0=ALU.mult, op1=ALU.mult)

                # FFN with Q folded into w2 (+ Q*diag(g1) residual) -> diff2 pair
                diff2 = ps1.tile([D, 2, N], F32, tag="diff2")

                def ffn(ln1, w1, w2q, qg, bf1, half):
                    for m in range(4):
                        hp = ps2.tile([D, 2, N], F32, tag="scratchA")
                        nc.tensor.matmul(hp[:, 0, :],
                                         w1[:, m * 128:(m + 1) * 128],
                                         ln1[:], start=True, stop=True)
                        hg = wk.tile([D, N], F32R, tag="hg")
                        nc.scalar.activation(hg[:], hp[:, 0, :], AF.Gelu,
                                             bias=bf1[:, m:m + 1])
                        nc.tensor.matmul(diff2[:, half, :], w2q[:, m, :],
                                         hg[:], start=(m == 0), stop=False)
                    nc.tensor.matmul(diff2[:, half, :], qg[:], ln1[:],
                                     start=False, stop=True)

                ffn(ln1p, w1p, w2p, qg1p, bf1p, 0)
                ffn(ln1t, w1t, w2t, qg1t, bf1t, 1)

                # biased centered pair, squares, var, r
                d2c = wk.tile([D, 2, N], F32R, tag="d2c")
                nc.vector.tensor_scalar_add(d2c[:, 0, :], diff2[:, 0, :], vcol(C_QC2P))
                nc.vector.tensor_scalar_add(d2c[:, 1, :], diff2[:, 1, :], vcol(C_QC2T))
                dsq2 = wk.tile([D, 2, N], F32R, tag="dsq2")
                nc.vector.scalar_tensor_tensor(
                    dsq2[:], d2c[:], 1.0, d2c[:], op0=ALU.mult, op1=ALU.mult)
                var2 = ps2.tile([D, 2, N], F32, tag="scratchA")
                nc.tensor.matmul(var2[:, 0, :], pm[:], dsq2[:, 0, :], start=True, stop=True)
                nc.tensor.matmul(var2[:, 1, :], pm[:], dsq2[:, 1, :], start=True, stop=True)
                r2 = wk.tile([D, 2, N], F32, tag="r2")
                nc.scalar.activation(r2[:], var2[:], AF.Abs_reciprocal_sqrt,
                                     bias=vcol(C_PAD))
                ln2 = wk.tile([D, 2, N], F32R, tag="ln2")
                nc.vector.tensor_tensor(ln2[:], d2c[:], r2[:], ALU.mult)

                # h1pre (g2 folded into wh1): materialize + stats on DVE
                h1_ps = ps2.tile([D, 2, N], F32, tag="scratchA")
                nc.tensor.matmul(h1_ps[:, 0, :], wh1p[:], ln2[:, 0, :],
                                 start=True, stop=False)
                nc.tensor.matmul(h1_ps[:, 0, :], wh1t[:], ln2[:, 1, :],
                                 start=False, stop=True)
                nc.vector.tensor_scalar(
                    h1pre[:, i, :], h1_ps[:, 0, :], vcol(C_BH1), 0.0,
                    op0=ALU.add, op1=ALU.add, accum_out=s1c[:, i:i + 1])
                sq = wk.tile([D, N], F32, tag="sq")
                nc.vector.scalar_tensor_tensor(
                    sq[:], h1pre[:, i, :].bitcast(F32), 1.0,
                    h1pre[:, i, :].bitcast(F32),
                    op0=ALU.mult, op1=ALU.mult, accum_out=s2c[:, i:i + 1])

            # ============ BN stats: reduce, allreduce, scale/shift ============
            def bn_stats(sc1, sc2, parts, g_ap, b_ap, eps_ap, tg):
                st = wk.tile([parts, 2], F32, tag=tg + "st")
                nc.vector.reduce_sum(st[:, 0:1], sc1[:], axis=mybir.AxisListType.X)
                nc.vector.reduce_sum(st[:, 1:2], sc2[:], axis=mybir.AxisListType.X)
                bin_t = dr.tile([parts, 2], F32, tag=tg + "i")
                bout_t = dr.tile([parts, 2], F32, tag=tg + "o")
                nc.sync.dma_start(bin_t[:], st[:])
                if single:
                    nc.sync.dma_start(bout_t[:], bin_t[:])
                else:
                    nc.gpsimd.collective_compute(
                        "AllReduce", ALU.add,
                        replica_groups=[list(range(N_CORES))],
                        ins=[bin_t.opt()], outs=[bout_t.opt()])
                g = wk.tile([parts, 2], F32, tag=tg + "g")
                nc.sync.dma_start(g[:], bout_t[:])
                mu = wk.tile([parts, 4], F32, tag=tg + "m")
                nc.vector.tensor_scalar_mul(mu[:, 0:2], g[:], 1.0 / B)  # mu | e
                nc.vector.tensor_tensor(mu[:, 2:3], mu[:, 0:1], mu[:, 0:1], ALU.mult)
                nc.vector.tensor_tensor(mu[:, 3:4], mu[:, 1:2], mu[:, 2:3],
                                        ALU.subtract)
                rb = wk.tile([parts, 3], F32, tag=tg + "r")
                nc.scalar.activation(rb[:, 0:1], mu[:, 3:4],
                                     AF.Abs_reciprocal_sqrt, bias=eps_ap)
                nc.vector.tensor_tensor(rb[:, 1:2], rb[:, 0:1], g_ap, ALU.mult)
                ms = wk.tile([parts, 1], F32, tag=tg + "x")
                nc.vector.tensor_tensor(ms[:], mu[:, 0:1], rb[:, 1:2], ALU.mult)
                nc.vector.tensor_tensor(rb[:, 2:3], b_ap, ms[:], ALU.subtract)
                return rb  # [:,1:2]=scale  [:,2:3]=shift

            bn1 = bn_stats(s1c, s2c, D, vcol(C_BN1G), vcol(C_BN1B), vcol(C_PAD), "bn1")

            # =================== phase C ===================
            for i in range(NT):
                h1g = wk.tile([D, N], F32R, tag="h1g")
                nc.scalar.activation(h1g[:], h1pre[:, i, :].bitcast(F32), AF.Gelu,
                                     scale=bn1[:, 1:2], bias=bn1[:, 2:3])
                h2_ps = ps1.tile([H2, 2, N], F32, tag="diff1")
                nc.tensor.matmul(h2_ps[:, 0, :], wh2[:], h1g[:], start=True, stop=True)
                nc.vector.tensor_scalar(
                    h2pre[:, i, :], h2_ps[:, 0, :], vech[:, 0:1], 0.0,
                    op0=ALU.add, op1=ALU.add, accum_out=u1c[:, i:i + 1])
                sq2 = wk.tile([H2, N], F32, tag="sq2")
                nc.vector.scalar_tensor_tensor(
                    sq2[:], h2pre[:, i, :].bitcast(F32), 1.0,
                    h2pre[:, i, :].bitcast(F32),
                    op0=ALU.mult, op1=ALU.mult, accum_out=u2c[:, i:i + 1])

            bn2 = bn_stats(u1c, u2c, H2, vech[:, 1:2], vech[:, 2:3], vech[:, 4:5], "bn2")

            # =================== phase E ===================
            for i in range(NT):
                h2g = wk.tile([H2, N], F32R, tag="h2g")
                nc.scalar.activation(h2g[:], h2pre[:, i, :].bitcast(F32), AF.Gelu,
                                     scale=bn2[:, 1:2], bias=bn2[:, 2:3])
                o_ps = ps1.tile([1, N], F32, tag="diff1")
                nc.tensor.matmul(o_ps[:], wout[:], h2g[:], start=True, stop=True)
                osb = wk.tile([1, N], F32, tag="osb")
                nc.scalar.activation(osb[:], o_ps[:], AF.Identity,
                                     bias=vech[0:1, 3:4])
                nc.sync.dma_start(y_d[:, i * N:(i + 1) * N], osb[:])

    nc.compile()
    return nc


def _prep_inputs(inputs):
    """Host-side: fold biases, transpose/pad x, build per-core in_maps."""
    f64 = lambda a: np.asarray(a, dtype=np.float64)
    x = np.asarray(inputs["x"], dtype=np.float32)

    w_pep, b_pep = f64(inputs["w_pep"]), f64(inputs["b_pep"])
    w_tcr, b_tcr = f64(inputs["w_tcr"]), f64(inputs["b_tcr"])
    wv_p2t, bv_p2t = f64(inputs["wv_p2t"]), f64(inputs["bv_p2t"])
    wo_p2t, bo_p2t = f64(inputs["wo_p2t"]), f64(inputs["bo_p2t"])
    wv_t2p, bv_t2p = f64(inputs["wv_t2p"]), f64(inputs["bv_t2p"])
    wo_t2p, bo_t2p = f64(inputs["wo_t2p"]), f64(inputs["bo_t2p"])

    W_ap = wo_p2t @ wv_p2t                  # pa_raw = W_ap @ tcr + c_ap
    c_ap = wo_p2t @ bv_p2t + bo_p2t
    W_at = wo_t2p @ wv_t2p
    c_at = wo_t2p @ bv_t2p + bo_t2p

    bias_z1p = b_pep + W_ap @ b_tcr + c_ap
    bias_z1t = b_tcr + W_at @ b_pep + c_at

    ffn_w1p, ffn_b1p = f64(inputs["ffn_w1p"]), f64(inputs["ffn_b1p"])
    ffn_w2p, ffn_b2p = f64(inputs["ffn_w2p"]), f64(inputs["ffn_b2p"])
    ffn_w1t, ffn_b1t = f64(inputs["ffn_w1t"]), f64(inputs["ffn_b1t"])
    ffn_w2t, ffn_b2t = f64(inputs["ffn_w2t"]), f64(inputs["ffn_b2t"])
    ln_b1p, ln_b1t = f64(inputs["ln_b1p"]), f64(inputs["ln_b1t"])
    ln_b2p, ln_b2t = f64(inputs["ln_b2p"]), f64(inputs["ln_b2t"])

    bias_f1p = ffn_w1p @ ln_b1p + ffn_b1p   # [512]
    bias_f1t = ffn_w1t @ ln_b1t + ffn_b1t
    q64 = np.eye(D) - np.full((D, D), 1.0 / D)
    qc2p = q64 @ (ffn_b2p + ln_b1p)         # Q @ (residual + ffn2 bias)
    qc2t = q64 @ (ffn_b2t + ln_b1t)

    w_h1, b_h1 = f64(inputs["w_h1"]), f64(inputs["b_h1"])
    bias_h1 = w_h1[:, :D] @ ln_b2p + w_h1[:, D:] @ ln_b2t + b_h1

    f32c = lambda a: np.ascontiguousarray(a, dtype=np.float32)
    ones = np.full((D, D), 1.0 / D, dtype=np.float32)
    qmat = np.eye(D, dtype=np.float32) - ones

    vecs = np.zeros((D, 12), dtype=np.float32)
    vecs[:, C_BZ1P] = bias_z1p
    vecs[:, C_BZ1T] = bias_z1t
    vecs[:, C_G1P] = inputs["ln_g1p"]
    vecs[:, C_G1T] = inputs["ln_g1t"]
    vecs[:, C_QC2P] = qc2p
    vecs[:, C_QC2T] = qc2t
    vecs[:, C_BH1] = bias_h1
    vecs[:, C_BN1G] = inputs["bn1_g"]
    vecs[:, C_BN1B] = inputs["bn1_b"]
    vecs[:, C_PAD] = EPS

    vech = np.zeros((H2, 5), dtype=np.float32)
    vech[:, 4] = EPS
    vech[:, 0] = inputs["b_h2"]
    vech[:, 1] = inputs["bn2_g"]
    vech[:, 2] = inputs["bn2_b"]
    vech[0, 3] = float(np.asarray(inputs["b_out"]).reshape(-1)[0])

    wtcr_pad = np.zeros((512, D), dtype=np.float32)
    wtcr_pad[:TCR, :] = f32c(w_tcr.T)

    common = {
        "wpepT": f32c(w_pep.T),
        "wtcrT": wtcr_pad,
        "wattnpT": f32c(W_ap.T),
        "wattntT": f32c(W_at.T),
        "qT": qmat,
        "pT": ones,
        "w1pT": f32c(ffn_w1p.T),
        "w1tT": f32c(ffn_w1t.T),
        "w2pT": f32c((q64 @ ffn_w2p).T),
        "w2tT": f32c((q64 @ ffn_w2t).T),
        "qg1pT": f32c(f64(inputs["ln_g1p"])[:, None] * q64),
        "qg1tT": f32c(f64(inputs["ln_g1t"])[:, None] * q64),
        "wh1pT": f32c(f64(inputs["ln_g2p"])[:, None] * w_h1[:, :D].T),
        "wh1tT": f32c(f64(inputs["ln_g2t"])[:, None] * w_h1[:, D:].T),
        "wh2T": f32c(f64(inputs["w_h2"]).T),
        "woutT": f32c(f64(inputs["w_out"]).T),
        "vecs": vecs,
        "vech": vech,
        "bf1p": f32c(bias_f1p.reshape(4, 128).T),
        "bf1t": f32c(bias_f1t.reshape(4, 128).T),
    }
    in_maps = []
    for c in range(N_CORES):
        xs = x[c * BC:(c + 1) * BC]         # [8192, 864]
        xt = np.zeros((XP, BC), dtype=np.float32)
        xt[:PEP + TCR, :] = xs.T
        m = dict(common)
        m["xt"] = xt
        in_maps.append(m)
    return in_maps


def kernel(**inputs) -> np.ndarray:
    global LAST_RESULT
    if "nc" not in _NC_CACHE:
        _NC_CACHE["nc"] = _build()
    nc = _NC_CACHE["nc"]
    in_maps = _prep_inputs(inputs)
    res = run_bass_kernel_spmd(nc, in_maps, core_ids=list(range(N_CORES)))
    LAST_RESULT = res
    out = np.concatenate([res.results[c]["y"].reshape(BC) for c in range(N_CORES)])
    return out.reshape(B, 1).astype(np.float32)


if __name__ == "__main__":
    import time
    t0 = time.time()
    nc = _build()
    print(f"build + bacc compile OK in {time.time() - t0:.1f}s")
    from concourse.bass_utils import compile_bass_kernel
    import tempfile
    t0 = time.time()
    neff = compile_bass_kernel(nc, tempfile.mkdtemp())
    print(f"walrus compile OK in {time.time() - t0:.1f}s -> {neff}")
